# revision 1
# baseline (speedup 1.0000x reference)
"""DifferentiableXGB forward on 8 TRN2 NeuronCores.

Data-parallel over batch: each core computes logits for its 4096-row slice.

Per-core device program (all matmuls in float32r, ~1e-4 relative precision):
  split_k^T [100t, b] = sum_d W_k[d, t]^T x^T[d, b]     (k = 0..3, PE, f32r)
  s^T = sum_k split_k^T + sum_k b1[:,k]                 (DVE chain + ACT bias)
  leaf_k^T = sigmoid(split_k^T + b1[:,k])               (ACT, per-partition bias)
  prod_k^T = leaf_k^T * s^T                             (DVE, f32r out)
  y^T [2, b] = sum_k g_k^T prod_k^T + fc_b              (PE accum + ACT bias)
where g_k[t, j] = fc_w[j, k] * final_weight[t].

The host passes x pre-transposed (so the contraction dim lands on SBUF
partitions with contiguous DMA) and transposes y back.
"""
import time
import numpy as np
from contextlib import ExitStack

N_CORES = 8
B, D, T, K = 32768, 1024, 100, 4
BL = B // N_CORES  # batch rows per core
NBT = BL // 512    # 512-wide batch tiles per core
ND = D // 128      # contraction chunks

_cache = {}


def build(reps=0):
    """Build + compile the per-core Bass program. reps>0 wraps the compute
    body in a HW loop executing it `reps` times (for steady-state timing)."""
    from concourse import bacc
    import concourse.mybir as mybir
    import concourse.tile as tile

    f32, f32r = mybir.dt.float32, mybir.dt.float32r
    AF = mybir.ActivationFunctionType

    nc = bacc.Bacc("TRN2", target_bir_lowering=False, debug=False)
    xt = nc.dram_tensor("xt", [D, BL], f32r, kind="ExternalInput")
    w = nc.dram_tensor("w", [ND, 128, K * T], f32r, kind="ExternalInput")
    biases = nc.dram_tensor("biases", [T, 8], f32, kind="ExternalInput")
    g = nc.dram_tensor("g", [T, 8], f32r, kind="ExternalInput")
    fcb = nc.dram_tensor("fcb", [2, 1], f32, kind="ExternalInput")
    y = nc.dram_tensor("y", [2, BL], f32, kind="ExternalOutput")

    with ExitStack() as ctx:
        tc = ctx.enter_context(tile.TileContext(nc))
        cp = ctx.enter_context(tc.tile_pool(name="const", bufs=1))
        wp = ctx.enter_context(tc.tile_pool(name="wp", bufs=1))
        xp = ctx.enter_context(tc.tile_pool(name="xp", bufs=1))
        ep = ctx.enter_context(tc.tile_pool(name="ep", bufs=3))
        sp = ctx.enter_context(tc.tile_pool(name="sp", bufs=7, space="PSUM"))
        op = ctx.enter_context(tc.tile_pool(name="op", bufs=1, space="PSUM"))

        bias_sb = cp.tile([T, 8], f32, name="bias_sb")
        nc.sync.dma_start(bias_sb[:], biases.ap())
        g_sb = cp.tile([T, 8], f32r, name="g_sb")
        nc.sync.dma_start(g_sb[:], g.ap())
        fcb_sb = cp.tile([2, 1], f32, name="fcb_sb")
        nc.sync.dma_start(fcb_sb[:], fcb.ap())
        out_sb = cp.tile([2, BL], f32, name="out_sb")

        ws = []
        for d in range(ND):
            wt = wp.tile([128, K * T], f32r, name=f"w{d}", tag=f"w{d}")
            nc.sync.dma_start(wt[:], w.ap()[d])
            ws.append(wt)
        xs = []
        for d in range(ND):
            xtile = xp.tile([128, BL], f32r, name=f"x{d}", tag=f"x{d}")
            xs.append(xtile)

        xap = xt.ap()

        def emit_body():
            # x loads, quarter-granule so compute overlaps the stream-in
            for q in range(4):
                qs = slice(q * (BL // 4), (q + 1) * (BL // 4))
                for d in range(ND):
                    nc.sync.dma_start(
                        xs[d][:, qs], xap[d * 128 : (d + 1) * 128, qs]
                    )
            for bt in range(NBT):
                sl = slice(bt * 512, (bt + 1) * 512)
                pst = [
                    sp.tile([T, 512], f32, name=f"split{bt}_{k}", tag="split")
                    for k in range(K)
                ]
                # last contraction chunk deferred so the in-order PE doesn't
                # head-of-line block on the quarter's final x DMA
                for k in range(K):
                    for d in range(ND - 1):
                        nc.tensor.matmul(
                            pst[k][:],
                            ws[d][:, k * T : (k + 1) * T],
                            xs[d][:, sl],
                            start=(d == 0),
                            stop=False,
                        )
                for k in range(K):
                    nc.tensor.matmul(
                        pst[k][:],
                        ws[ND - 1][:, k * T : (k + 1) * T],
                        xs[ND - 1][:, sl],
                        start=False,
                        stop=True,
                    )
                s1 = ep.tile([T, 512], f32, name=f"s1_{bt}", tag="s1")
                nc.vector.tensor_copy(s1[:], pst[0][:])
                s2 = ep.tile([T, 512], f32, name=f"s2_{bt}", tag="s2")
                nc.vector.tensor_add(s2[:], s1[:], pst[1][:])
                s3 = ep.tile([T, 512], f32, name=f"s3_{bt}", tag="s3")
                nc.vector.tensor_add(s3[:], s2[:], pst[2][:])
                s4 = ep.tile([T, 512], f32, name=f"s4_{bt}", tag="s4")
                nc.vector.tensor_add(s4[:], s3[:], pst[3][:])
                s = ep.tile([T, 512], f32, name=f"s_{bt}", tag="s")
                nc.scalar.activation(s[:], s4[:], AF.Identity, bias=bias_sb[:, 4:5])

                ps2 = op.tile([2, 512], f32, name=f"ps2_{bt}", tag="ps2")
                for k in range(K):
                    leaf = ep.tile([T, 512], f32, name=f"leaf{bt}_{k}", tag="leaf", bufs=4)
                    nc.scalar.activation(
                        leaf[:], pst[k][:], AF.Sigmoid, bias=bias_sb[:, k : k + 1]
                    )
                    prod = ep.tile([T, 512], f32r, name=f"prod{bt}_{k}", tag="prod", bufs=4)
                    nc.vector.tensor_mul(prod[:], leaf[:], s[:])
                    nc.tensor.matmul(
                        ps2[:],
                        g_sb[:, k * 2 : k * 2 + 2],
                        prod[:],
                        start=(k == 0),
                        stop=(k == K - 1),
                    )
                nc.scalar.activation(
                    out_sb[:, sl], ps2[:], AF.Identity, bias=fcb_sb[:]
                )

        if reps > 0:
            with tc.For_i(0, reps, 1):
                emit_body()
        else:
            emit_body()

        nc.sync.dma_start(y.ap(), out_sb[:])
    nc.compile()
    return nc


def make_in_maps(x, W1, b1, final_weight, fc_w, fc_b):
    x = np.asarray(x, np.float32)
    W1 = np.asarray(W1, np.float32)
    b1 = np.asarray(b1, np.float32)
    final_weight = np.asarray(final_weight, np.float32)
    fc_w = np.asarray(fc_w, np.float32)
    fc_b = np.asarray(fc_b, np.float32)

    xtT = np.ascontiguousarray(x.T)  # [D, B]
    w8 = np.ascontiguousarray(
        W1.transpose(2, 1, 0).reshape(ND, 128, K * T)
    )  # [d-chunk, p, k*T + t]
    bias_mat = np.zeros((T, 8), np.float32)
    bias_mat[:, :K] = b1
    bias_mat[:, K] = b1.sum(1)
    g_mat = np.zeros((T, 8), np.float32)
    for k in range(K):
        for j in range(2):
            g_mat[:, k * 2 + j] = fc_w[j, k] * final_weight
    fcb_mat = np.ascontiguousarray(fc_b.reshape(2, 1))

    in_maps = []
    for c in range(N_CORES):
        in_maps.append(
            {
                "xt": np.ascontiguousarray(xtT[:, c * BL : (c + 1) * BL]),
                "w": w8,
                "biases": bias_mat,
                "g": g_mat,
                "fcb": fcb_mat,
            }
        )
    return in_maps


def kernel(x, W1, b1, final_weight, fc_w, fc_b):
    from concourse.bass_utils import run_bass_kernel_spmd

    if "nc" not in _cache:
        _cache["nc"] = build()
    nc = _cache["nc"]
    in_maps = make_in_maps(x, W1, b1, final_weight, fc_w, fc_b)

    last_err = None
    for attempt in range(3):
        try:
            res = run_bass_kernel_spmd(nc, in_maps, core_ids=list(range(N_CORES)))
            break
        except Exception as e:  # transient device wedge: wait for recovery
            last_err = e
            time.sleep(90)
    else:
        raise last_err

    out = np.empty((B, 2), np.float32)
    for c in range(N_CORES):
        out[c * BL : (c + 1) * BL, :] = res.results[c]["y"].T
    return out



# revision 2
# speedup vs baseline: 1.4376x; 1.4376x over previous
"""DifferentiableXGB forward on 8 TRN2 NeuronCores.

Data-parallel over batch; per-core batch slice BL=4096.

Layout: batch on PSUM partitions (x tile is the matmul stationary operand),
tree*leaf (400) on the free axis — the PE array runs at full 128-wide
utilization (102400 stream cycles/core vs 131072 for the t-on-partitions
layout, with no weight-reload exposure since each LDWEIGHTS hides under the
previous 400-cycle stream).

Per batch-group g (4 tiles of 128 rows, psum [128, 4x512]):
  psum[b, k*100+t] = sum_d x[b,d] W1[t,k,d]          (PE, fp16 in / f32 acc)
  splitb = psum + b1                                 (DVE, evacuate to f16)
  leaf   = sigmoid(splitb)                           (ACT)
  s1     = (sum_k splitb) * final_weight[t]          (GpSimd, 3 ops)
  prod   = leaf * s1                                 (DVE)
  q[b,(bt,k)] = sum_t prod                           (DVE halve + reduce)
Final y = q @ fc_w.T + fc_b is 0.26 MFLOP — done on host after gather.

x is streamed from HBM every rep (ping-pong buffered across the hardware
reps loop); fp16 halves the HBM traffic vs f32.
"""
import time
import numpy as np
from contextlib import ExitStack

N_CORES = 8
B, D, T, K = 32768, 1024, 100, 4
BL = B // N_CORES   # batch rows per core
ND = D // 128       # contraction chunks
NBT = BL // 128     # 128-row batch tiles per core (32)
NG = NBT // 4       # groups of 4 batch tiles (8)
C = K * T           # free-axis width per batch tile (400)

_cache = {}


def build(reps=0):
    """Build + compile the per-core Bass program. reps>0 executes the
    rep body `reps` times (reps must be odd; hardware loop runs pairs
    with ping-pong x buffers, plus one epilogue body)."""
    from concourse import bacc
    import concourse.mybir as mybir
    import concourse.tile as tile

    f32, f16 = mybir.dt.float32, mybir.dt.float16
    AF = mybir.ActivationFunctionType
    ALU = mybir.AluOpType
    AX = mybir.AxisListType

    nc = bacc.Bacc("TRN2", target_bir_lowering=False, debug=False)
    xt = nc.dram_tensor("xt", [ND, 128, BL], f16, kind="ExternalInput")
    w = nc.dram_tensor("w", [ND, 128, C], f16, kind="ExternalInput")
    biasb = nc.dram_tensor("biasb", [128, C], f32, kind="ExternalInput")
    fwb = nc.dram_tensor("fwb", [128, T], f16, kind="ExternalInput")
    q = nc.dram_tensor("q", [128, NBT * K], f32, kind="ExternalOutput")

    with ExitStack() as ctx:
        tc = ctx.enter_context(tile.TileContext(nc))
        cp = ctx.enter_context(tc.tile_pool(name="const", bufs=1))
        xp = ctx.enter_context(tc.tile_pool(name="xp", bufs=1))
        ep = ctx.enter_context(tc.tile_pool(name="ep", bufs=2))
        sp = ctx.enter_context(tc.tile_pool(name="sp", bufs=2, space="PSUM"))

        biasb_sb = cp.tile([128, C], f32, name="biasb_sb")
        nc.sync.dma_start(biasb_sb[:], biasb.ap())
        fwb_sb = cp.tile([128, T], f16, name="fwb_sb")
        nc.sync.dma_start(fwb_sb[:], fwb.ap())
        qall = cp.tile([128, NBT * K], f32, name="qall")

        ws = []
        for d in range(ND):
            wt = cp.tile([128, C], f16, name=f"w{d}")
            nc.sync.dma_start(wt[:], w.ap()[d])
            ws.append(wt)

        # two x buffer sets for cross-rep ping-pong
        xs = [
            [xp.tile([128, BL], f16, name=f"x{s}_{d}") for d in range(ND)]
            for s in range(2)
        ]
        xap = xt.ap()

        def dma_x(s):
            for d in range(ND):
                nc.sync.dma_start(xs[s][d][:], xap[d])

        bias_bc = biasb_sb[:].unsqueeze(1).broadcast_to([128, 4, C])
        fw_bc = (
            fwb_sb[:].unsqueeze(1).broadcast_to([128, 4, T])
        )

        def compute(s):
            x_ = xs[s]
            for g in range(NG):
                psg = sp.tile([128, 4 * 512], f32, name=f"ps{s}_{g}", tag="psg")
                for bl in range(4):
                    bt = g * 4 + bl
                    for d in range(ND):
                        nc.tensor.matmul(
                            psg[:, bl * 512 : bl * 512 + C],
                            x_[d][:, bt * 128 : (bt + 1) * 128],
                            ws[d][:],
                            start=(d == 0),
                            stop=(d == ND - 1),
                        )
                ps4 = psg[:].rearrange("p (bl c) -> p bl c", bl=4, c=512)[
                    :, :, 0:C
                ]
                splitb = ep.tile([128, 4 * C], f16, name=f"sb{s}_{g}", tag="splitb")
                sb4 = splitb[:].rearrange("p (bl c) -> p bl c", bl=4, c=C)
                nc.vector.tensor_add(sb4, ps4, bias_bc)

                leaf = ep.tile([128, 4 * C], f16, name=f"lf{s}_{g}", tag="leaf")
                nc.scalar.activation(leaf[:], splitb[:], AF.Sigmoid)

                sbk = splitb[:].rearrange(
                    "p (bl k t) -> p bl k t", bl=4, k=K, t=T
                )
                s2 = ep.tile([128, 4 * 2 * T], f16, name=f"s2_{s}_{g}", tag="s2")
                s2v = s2[:].rearrange("p (bl k t) -> p bl k t", bl=4, k=2, t=T)
                nc.gpsimd.tensor_add(s2v, sbk[:, :, 0:2, :], sbk[:, :, 2:4, :])
                s3 = ep.tile([128, 4 * T], f16, name=f"s3_{s}_{g}", tag="s3")
                s3v = s3[:].rearrange("p (bl t) -> p bl t", bl=4, t=T)
                nc.gpsimd.tensor_add(s3v, s2v[:, :, 0, :], s2v[:, :, 1, :])
                s1 = ep.tile([128, 4 * T], f16, name=f"s1_{s}_{g}", tag="s1")
                s1v = s1[:].rearrange("p (bl t) -> p bl t", bl=4, t=T)
                nc.gpsimd.tensor_mul(s1v, s3v, fw_bc)

                prod = ep.tile([128, 4 * C], f16, name=f"pr{s}_{g}", tag="prod")
                prv = prod[:].rearrange(
                    "p (bl k t) -> p bl k t", bl=4, k=K, t=T
                )
                lfv = leaf[:].rearrange(
                    "p (bl k t) -> p bl k t", bl=4, k=K, t=T
                )
                s1b = (
                    s1[:]
                    .rearrange("p (bl t) -> p bl t", bl=4, t=T)
                    .unsqueeze(2)
                    .broadcast_to([128, 4, K, T])
                )
                nc.vector.tensor_mul(prv, lfv, s1b)

                ph = ep.tile([128, 4 * K * 50], f16, name=f"ph{s}_{g}", tag="ph")
                phv = ph[:].rearrange(
                    "p (bl k t) -> p bl k t", bl=4, k=K, t=50
                )
                prh = prod[:].rearrange(
                    "p (bl k h t) -> p bl k h t", bl=4, k=K, h=2, t=50
                )
                nc.vector.tensor_add(phv, prh[:, :, :, 0, :], prh[:, :, :, 1, :])

                qv = qall[:, g * 16 : (g + 1) * 16].rearrange(
                    "p (bl k) -> p bl k", bl=4, k=K
                )
                nc.vector.tensor_reduce(qv, phv, axis=AX.X, op=ALU.add)

        dma_x(0)
        if reps > 1:
            n_pairs = (reps - 1) // 2
            assert reps == 2 * n_pairs + 1, "reps must be odd"
            with tc.For_i(0, n_pairs, 1):
                dma_x(1)
                compute(0)
                dma_x(0)
                compute(1)
        compute(0)

        nc.sync.dma_start(q.ap(), qall[:])
    nc.compile()
    return nc


def make_in_maps(x, W1, b1, final_weight, fc_w, fc_b):
    x = np.asarray(x, np.float32)
    W1 = np.asarray(W1, np.float32)
    b1 = np.asarray(b1, np.float32)
    final_weight = np.asarray(final_weight, np.float32)

    w8 = np.ascontiguousarray(
        W1.transpose(2, 1, 0).reshape(ND, 128, C).astype(np.float16)
    )
    biasb = np.ascontiguousarray(
        np.broadcast_to(b1.T.reshape(1, C), (128, C)).astype(np.float32)
    )
    fwb = np.ascontiguousarray(
        np.broadcast_to(
            final_weight.reshape(1, T).astype(np.float16), (128, T)
        )
    )

    in_maps = []
    for c in range(N_CORES):
        xc = x[c * BL : (c + 1) * BL, :].T.astype(np.float16)  # [D, BL]
        in_maps.append(
            {
                "xt": np.ascontiguousarray(xc.reshape(ND, 128, BL)),
                "w": w8,
                "biasb": biasb,
                "fwb": fwb,
            }
        )
    return in_maps


def finish(q_per_core, fc_w, fc_b):
    """Host epilogue: q [128, 32*4] per core -> y [B, 2]."""
    fc_w = np.asarray(fc_w, np.float32)
    fc_b = np.asarray(fc_b, np.float32)
    out = np.empty((B, 2), np.float32)
    for c in range(N_CORES):
        qc = np.asarray(q_per_core[c], np.float32).reshape(128, NBT, K)
        yc = np.einsum("pbk,jk->bpj", qc, fc_w).reshape(BL, 2) + fc_b
        out[c * BL : (c + 1) * BL] = yc
    return out


def kernel(x, W1, b1, final_weight, fc_w, fc_b):
    from concourse.bass_utils import run_bass_kernel_spmd

    if "nc" not in _cache:
        _cache["nc"] = build()
    nc = _cache["nc"]
    in_maps = make_in_maps(x, W1, b1, final_weight, fc_w, fc_b)

    last_err = None
    for attempt in range(3):
        try:
            res = run_bass_kernel_spmd(nc, in_maps, core_ids=list(range(N_CORES)))
            break
        except Exception as e:  # transient device wedge: wait for recovery
            last_err = e
            time.sleep(90)
    else:
        raise last_err

    return finish([res.results[c]["q"] for c in range(N_CORES)], fc_w, fc_b)
